# revision 22
# baseline (speedup 1.0000x reference)
"""Multi-head attention (B=2, S=2048, D=1024, H=16) on 8 trn2 NeuronCores.

Sharding: tensor-parallel over heads (2 heads/core). Each core computes
q/k/v projections for its head slice (contraction over D done on-device,
including the x transpose via the PE), runs attention for its heads over
both batches, then an 8-way AllToAll re-shards the attention output by
sequence rows so each core computes a disjoint 512-row slice of the final
output projection. Host only slices/concatenates and pre-transposes weight
layouts.

Matmuls run in float32r (full PE rate at moving-dim >= 256). Measured on
HW: mixing K=64 and K=128 matmuls in the PE stream degrades every matmul
to ~724ns (vs 227ns uniform), so the per-head d_k=64 scores contraction is
zero-padded to K=128: kT is stored per-head with the other head's 64
partition rows zeroed, letting the full 128-row qT be used as the moving
operand (the zero rows kill the cross-head terms).

v is stored per j-tile as [v_h0 | ones | v_h1] (192 cols): head h's attn@v
lhsT is the contiguous 128-col slice starting at 64*h, so rows of the
output hold {h0: raw 0:64, denom 64:128} and {h1: denom 0:64, raw 64:128}
— the shared ones block gives every partition a replicated softmax
denominator without doubling v's footprint.
"""

import numpy as np

import concourse.bass as bass
import concourse.mybir as mybir
import concourse.tile as tile
from concourse import bacc
from concourse.masks import make_identity
from concourse.bass_utils import run_bass_kernel_spmd

# problem constants (hardcoded per harness contract)
B, S, D = 2, 2048, 1024
H, DK = 16, 64
NCORES = 8
HPC = H // NCORES          # heads per core = 2
CS = HPC * DK              # per-core channel slice = 128
T = B * S                  # 4096 total rows
TCH = 512                  # stage-A t-chunk
NTCH = T // TCH            # 8
IB = T // NCORES           # 512 output rows per core
P = 128
F32 = mybir.dt.float32
F32R = mybir.dt.float32r
AF = mybir.ActivationFunctionType
ALU = mybir.AluOpType


def build_nc():
    nc = bacc.Bacc("TRN2", target_bir_lowering=False, debug=False, num_devices=NCORES)

    x = nc.dram_tensor("x", [T, D], F32, kind="ExternalInput")
    wqT = nc.dram_tensor("wqT", [D, CS], F32R, kind="ExternalInput")
    wkT = nc.dram_tensor("wkT", [D, CS], F32R, kind="ExternalInput")
    wvT = nc.dram_tensor("wvT", [D, CS], F32R, kind="ExternalInput")
    woT = nc.dram_tensor("woT", [D, D], F32R, kind="ExternalInput")
    bq = nc.dram_tensor("bq", [CS, 1], F32, kind="ExternalInput")
    bk = nc.dram_tensor("bk", [CS, 1], F32, kind="ExternalInput")
    bv = nc.dram_tensor("bv", [CS, 1], F32, kind="ExternalInput")
    bo = nc.dram_tensor("bo", [1, D], F32R, kind="ExternalInput")
    y = nc.dram_tensor("y", [IB, D], F32, kind="ExternalOutput")

    with tile.TileContext(nc) as tc:
        with (
            tc.tile_pool(name="const", bufs=1) as cpool,
            tc.tile_pool(name="persist", bufs=1) as ppool,
            tc.tile_pool(name="dram", bufs=1, space="DRAM") as dpool,
        ):
            ident = cpool.tile([P, P], F32)
            make_identity(nc, ident[:])

            bq_sb = cpool.tile([CS, 1], F32)
            bk_sb = cpool.tile([CS, 1], F32)
            bv_sb = cpool.tile([CS, 1], F32)
            bo_sb = cpool.tile([1, D], F32R)
            nc.gpsimd.dma_start(bq_sb[:], bq[:])
            nc.gpsimd.dma_start(bk_sb[:], bk[:])
            nc.gpsimd.dma_start(bv_sb[:], bv[:])
            nc.gpsimd.dma_start(bo_sb[:], bo[:])

            ones32 = cpool.tile([P, 512], F32)
            nc.vector.memset(ones32[:], 1.0)
            zeros32 = cpool.tile([P, 512], F32)
            nc.vector.memset(zeros32[:], 0.0)
            ones_row = cpool.tile([1, P], F32R)
            nc.vector.tensor_copy(ones_row[:], ones32[0:1, 0:1].to_broadcast([1, P]))

            # bo broadcast to all partitions via two K=1 matmuls, done once
            # up-front so no K=1 matmul pollutes the K=128 streams later
            bo_full = cpool.tile([P, D], F32)
            with tc.tile_pool(name="psbo", bufs=1, space="PSUM") as psbo:
                for nch in range(D // 512):
                    pb = psbo.tile([P, 512], F32, tag="pb")
                    nc.tensor.matmul(
                        pb[:], ones_row[:], bo_sb[:, nch * 512:(nch + 1) * 512],
                        start=True, stop=True,
                    )
                    nc.vector.tensor_copy(bo_full[:, nch * 512:(nch + 1) * 512], pb[:])

            # persistent per-core activations, split per batch so stage B
            # chunks for batch 0 can start while stage A still fills batch 1
            qTb = [ppool.tile([CS, S], F32R, name=f"qT{b}") for b in range(B)]
            kThb = [[ppool.tile([CS, S], F32R, name=f"kTh{b}_{h}")
                     for h in range(HPC)] for b in range(B)]
            for b in range(B):
                for h in range(HPC):
                    z0 = (1 - h) * DK  # zero rows: h=0 -> 64:128, h=1 -> 0:64
                    nc.vector.tensor_copy(
                        kThb[b][h][z0:z0 + DK, :],
                        zeros32[z0:z0 + DK, None, :].to_broadcast(
                            [DK, S // 512, 512]),
                    )
            # v per j-tile as [v_h0 | ones | v_h1] (see module docstring)
            v_sbb = [ppool.tile([P, S // P, 3 * DK], F32R, name=f"v{b}")
                     for b in range(B)]
            for b in range(B):
                nc.vector.tensor_copy(
                    v_sbb[b][:, :, DK:2 * DK],
                    ones32[:, None, 0:DK].to_broadcast([P, S // P, DK]),
                )

            # ---- stage A: x transpose + q/k/v projections ----
            with (
                tc.tile_pool(name="aw", bufs=1) as awpool,
                tc.tile_pool(name="stageA", bufs=2) as apool,
                tc.tile_pool(name="psA", bufs=2, space="PSUM") as psA,
                tc.tile_pool(name="psP", bufs=3, space="PSUM") as psP,
            ):
                # weights/biases go over the gpsimd SWDGE queue so they do
                # not delay the x stream on the sync HWDGE queue
                wq_sb = awpool.tile([P, 8, CS], F32R)
                wk_sb = awpool.tile([P, 8, CS], F32R)
                wv_sb = awpool.tile([P, 8, CS], F32R)
                nc.gpsimd.dma_start(wq_sb[:], wqT[:].rearrange("(o p) c -> p o c", p=P))
                nc.gpsimd.dma_start(wk_sb[:], wkT[:].rearrange("(o p) c -> p o c", p=P))
                nc.gpsimd.dma_start(wv_sb[:], wvT[:].rearrange("(o p) c -> p o c", p=P))
                vT = awpool.tile([CS, T], F32)
                for te in range(NTCH):
                    x_e = apool.tile([P, TCH // P, D], F32, tag="x_e")
                    if te == 0:
                        # split the first load so transposes start sooner
                        for hh in range(4):
                            r0 = hh * (TCH // 4)
                            nc.sync.dma_start(
                                x_e[:, hh:hh + 1, :],
                                x[r0:r0 + TCH // 4, :].rearrange(
                                    "(tt p) d -> p tt d", p=P
                                ),
                            )
                    else:
                        nc.sync.dma_start(
                            x_e[:],
                            x[te * TCH:(te + 1) * TCH, :].rearrange(
                                "(tt p) d -> p tt d", p=P
                            ),
                        )
                    xT_e = apool.tile([P, 8, TCH], F32R, tag="xT_e")
                    for dd in range(8):
                        ps = psA.tile([P, TCH], F32, tag="trps")
                        for tt in range(TCH // P):
                            nc.tensor.transpose(
                                ps[:, tt * P:(tt + 1) * P],
                                x_e[:, tt, dd * P:(dd + 1) * P],
                                ident[:],
                            )
                        nc.vector.tensor_copy(xT_e[:, dd, :], ps[:])
                    for proj, (w_sb, b_sb) in enumerate(
                        ((wq_sb, bq_sb), (wk_sb, bk_sb), (wv_sb, bv_sb))
                    ):
                        pp = psP.tile([CS, TCH], F32, tag="projps")
                        for dd in range(8):
                            nc.tensor.matmul(
                                pp[:],
                                w_sb[:, dd, :],
                                xT_e[:, dd, :],
                                start=(dd == 0),
                                stop=(dd == 7),
                            )
                        tb = te // (NTCH // B)      # batch of this t-chunk
                        sl = slice((te % (NTCH // B)) * TCH,
                                   (te % (NTCH // B) + 1) * TCH)
                        if proj == 0:
                            nc.vector.tensor_tensor(
                                qTb[tb][:, sl], pp[:],
                                b_sb[:].to_broadcast([CS, TCH]), ALU.add,
                            )
                        elif proj == 1:
                            for h in range(HPC):
                                hs = slice(h * DK, (h + 1) * DK)
                                nc.vector.tensor_tensor(
                                    kThb[tb][h][hs, sl], pp[hs, :],
                                    b_sb[hs, :].to_broadcast([DK, TCH]), ALU.add,
                                )
                        else:
                            nc.vector.tensor_tensor(
                                vT[:, sl.start + tb * S:sl.stop + tb * S], pp[:],
                                b_sb[:].to_broadcast([CS, TCH]), ALU.add,
                            )
                    if te % (NTCH // B) == NTCH // B - 1:
                        # batch tb fully projected: transpose its v now so
                        # stage B chunks for this batch are unblocked
                        for tt in range(S // P):
                            gt = tb * (S // P) + tt
                            psv = psA.tile([P, P], F32, tag="vtr")
                            nc.tensor.transpose(
                                psv[:], vT[:, gt * P:(gt + 1) * P], ident[:])
                            nc.vector.tensor_copy(
                                v_sbb[tb][:, tt, 0:DK], psv[:, 0:DK])
                            nc.vector.tensor_copy(
                                v_sbb[tb][:, tt, 2 * DK:3 * DK],
                                psv[:, DK:2 * DK])

            # ---- stage C tiles allocated now (reuse stage A's space);
            # the 4 MB wo load overlaps stage B compute on the sync queue
            with (
                tc.tile_pool(name="stageC", bufs=1) as c2pool,
                tc.tile_pool(name="yout", bufs=2) as ypool,
            ):
                wo_sb = c2pool.tile([P, 8, D], F32R)
                nc.sync.dma_start(wo_sb[:], woT[:].rearrange("(o p) n -> p o n", p=P))
                h_sb = c2pool.tile([P, 8, IB], F32R)

                # ---- stage B: attention per output i-chunk ----
                a2a_in = dpool.tile([NCORES, CS, IB], F32R)
                a2a_out = dpool.tile([NCORES, CS, IB], F32R)
                with (
                    tc.tile_pool(name="et", bufs=7) as etpool,
                    tc.tile_pool(name="ob", bufs=6) as obpool,
                    tc.tile_pool(name="psS", bufs=3, space="PSUM") as psS,
                    tc.tile_pool(name="psAV", bufs=1, space="PSUM") as psAV,
                ):
                    for g in range(NCORES):
                        b = g // (NCORES // B)     # batch of this i-chunk
                        i0 = (g % (NCORES // B)) * IB
                        av_ps = [
                            psAV.tile([P, IB], F32, tag=f"av{h}", name=f"av{h}")
                            for h in range(HPC)
                        ]
                        for jt in range(S // P):
                            j0 = jt * P
                            sps = psS.tile([P, HPC * IB], F32, tag="s")
                            for h in range(HPC):
                                nc.tensor.matmul(
                                    sps[:, h * IB:(h + 1) * IB],
                                    kThb[b][h][:, j0:j0 + P],
                                    qTb[b][:, i0:i0 + IB],
                                    start=True,
                                    stop=True,
                                )
                            et = etpool.tile([P, HPC * IB], F32R, tag="et")
                            nc.scalar.activation(et[:], sps[:], AF.Exp, scale=0.125)
                            for h in range(HPC):
                                nc.tensor.matmul(
                                    av_ps[h][:],
                                    v_sbb[b][:, jt, h * DK:h * DK + 2 * DK],
                                    et[:, h * IB:(h + 1) * IB],
                                    start=(jt == 0),
                                    stop=(jt == S // P - 1),
                                )
                        for h in range(HPC):
                            # one fast copy frees the accumulator bank; the
                            # reciprocal+normalize run off the PE path.
                            # h0: raw rows 0:64, denom rows 64:128 (flipped
                            # for h1 — its lhsT starts with the ones block)
                            raw = obpool.tile([P, IB], F32, tag="raw")
                            nc.vector.tensor_copy(raw[:], av_ps[h][:])
                            raw_sl = slice(0, DK) if h == 0 else slice(DK, 2 * DK)
                            den_sl = slice(DK, 2 * DK) if h == 0 else slice(0, DK)
                            rec = obpool.tile([P, IB], F32, tag="rec")
                            nc.vector.reciprocal(rec[raw_sl, :], raw[den_sl, :])
                            onrm = obpool.tile([P, IB], F32, tag="onrm")
                            nc.vector.tensor_tensor(
                                onrm[raw_sl, :],
                                raw[raw_sl, :],
                                rec[raw_sl, :],
                                ALU.mult,
                            )
                            nc.sync.dma_start(
                                a2a_in[g, h * DK:(h + 1) * DK, :],
                                onrm[raw_sl, :].bitcast(F32R),
                            )

                nc.gpsimd.collective_compute(
                    "AllToAll",
                    ALU.bypass,
                    replica_groups=[list(range(NCORES))],
                    ins=[a2a_in.opt()],
                    outs=[a2a_out.opt()],
                )

                # ---- stage C: output projection for my 512 rows ----
                with tc.tile_pool(name="psY", bufs=2, space="PSUM") as psY:
                    # per-source-core loads so the first matmuls can start
                    # before the whole gather lands
                    for o in range(8):
                        nc.sync.dma_start(h_sb[:, o, :], a2a_out[o, :, :])
                    for it in range(IB // P):
                        y_sb = ypool.tile([P, D], F32, tag="y")
                        for nch in range(D // 512):
                            py = psY.tile([P, 512], F32, tag="py")
                            for o in range(8):
                                nc.tensor.matmul(
                                    py[:],
                                    h_sb[:, o, it * P:(it + 1) * P],
                                    wo_sb[:, o, nch * 512:(nch + 1) * 512],
                                    start=(o == 0),
                                    stop=(o == 7),
                                )
                            nc.vector.tensor_tensor(
                                y_sb[:, nch * 512:(nch + 1) * 512],
                                py[:],
                                bo_full[:, nch * 512:(nch + 1) * 512],
                                ALU.add,
                            )
                        nc.sync.dma_start(
                            y[it * P:(it + 1) * P, :], y_sb[:]
                        )

    nc.compile()
    return nc


_NC = None


def _get_nc():
    global _NC
    if _NC is None:
        _NC = build_nc()
    return _NC


def _make_in_maps(x, Wq, bq, Wk, bk, Wv, bv, Wo, bo):
    xf = np.ascontiguousarray(np.asarray(x, np.float32).reshape(T, D))
    woT = np.ascontiguousarray(np.asarray(Wo, np.float32).T)
    bo_r = np.ascontiguousarray(np.asarray(bo, np.float32).reshape(1, D))
    Wq = np.asarray(Wq, np.float32)
    Wk = np.asarray(Wk, np.float32)
    Wv = np.asarray(Wv, np.float32)
    in_maps = []
    for c in range(NCORES):
        sl = slice(c * CS, (c + 1) * CS)
        in_maps.append({
            "x": xf,
            "wqT": np.ascontiguousarray(Wq[sl, :].T),
            "wkT": np.ascontiguousarray(Wk[sl, :].T),
            "wvT": np.ascontiguousarray(Wv[sl, :].T),
            "woT": woT,
            "bq": np.ascontiguousarray(np.asarray(bq, np.float32)[sl]).reshape(CS, 1),
            "bk": np.ascontiguousarray(np.asarray(bk, np.float32)[sl]).reshape(CS, 1),
            "bv": np.ascontiguousarray(np.asarray(bv, np.float32)[sl]).reshape(CS, 1),
            "bo": bo_r,
        })
    return in_maps


def _assemble(results):
    yfull = np.concatenate([results[c]["y"] for c in range(NCORES)], axis=0)
    return yfull.reshape(B, S, D)


def run_traced(trace=False, **inputs):
    """Run and return (output, BassKernelResults) — used by test.py."""
    nc = _get_nc()
    res = run_bass_kernel_spmd(
        nc, _make_in_maps(**inputs), core_ids=list(range(NCORES)), trace=trace
    )
    return _assemble(res.results), res


def kernel(**inputs) -> np.ndarray:
    out, _ = run_traced(trace=False, **inputs)
    return out
